# revision 20
# baseline (speedup 1.0000x reference)
"""AFPM (adaptive per-patch modulation) kernel for 8 TRN2 NeuronCores.

Reference computation (B=8, C=64, H=W=512, K=8, HID=64):
  - d[l]: normalized distance of each 8x8 patch center from image center
  - pk[l, kk] / pb[l]: tiny MLPs of d (host-precomputable, data-independent)
  - feats[b,c,l] = sum_kk patches[b,c,kk,l] * pk[l,kk] + pb[l]
  - feats2 = conv_w @ feats + conv_b           (1x1 conv over channels)
  - out patches = patches * feats2[:, :, None, :]

Sharding: core i handles patch-rows i*8..i*8+7 for ALL 8 images.

v12 layout: partitions = (dy2, c) -- the TOP/BOTTOM HALF of each patch
rides the partition axis instead of an image pair.  The dy2-sum then
happens inside the PE conv contraction for free (stationary becomes
tile(conv_w.T, (2,2)), summing both partition halves into both output
halves), deleting one DVE tree level.  unit = (row-pair tp, image b),
tp-major; 32 units of [128, free 4096 = (t2, dyf4, pw64, dx8)], 1 MiB
DMA each direction.  HBM floor = 64 MiB / 358 GB/s ~= 187 us.

  DMA  in   : 1 MiB, rings alternate sync/scalar per unit
  DVE  mul  : PROD = xb * PKREP[tp]     flat TT, 2x bf16   (~2.2us)
  DVE  L1   : dyf 4->2 halving add                         (~1.2us)
  DVE  L2   : dyf 2->1, writes into the unit's half of a shared
              pair tile T3 [128, (b2 t2 pw dx)]            (~0.63us)
  PE   g    : per unit-PAIR, 8 accumulating strided matmuls n=256
              over dx slices of T3 with stationary bd2
              (dy2-sum + dx-sum + conv all in one PSUM group)
              (+) w2.T @ [pb; 1]
  ACT  gexp : bf16(g) expanded over dx  [128, 2048] per pair
  DVE  out  : OUT(prod buf) = xb * bcast(gexp), deferred ~4 units
  DMA  out  : 1 MiB from the prod buffer on the opposite ring

NOTE: gpsimd is deliberately UNUSED for tensor work -- Pool TT ops
contend with DVE for SBUF ports and inflate DVE ops 35-90% (measured),
a net loss.  PE/ACT traffic does not measurably slow DVE.

pk tables: only 4 (one per row-pair, shared by all 8 images).  They
ship as [4, 2, 4096] bf16 half-lines; on-device a 2-row ones matmul
broadcasts line dy2 to partition half dy2 (PE -> PSUM -> ACT copy).
Row-pair 0 additionally ships pre-replicated (1 MiB, scalar ring at
t=0) so the first mul isn't gated on the broadcast chain.
"""

import math
import sys

import numpy as np

for _p in ("/opt/trn_rl_repo",):
    if _p not in sys.path:
        sys.path.insert(0, _p)

import concourse.bass as bass
import concourse.tile as tile
from concourse import bacc, mybir
from concourse.bass_utils import run_bass_kernel_spmd

B, C, H, W, K, HID = 8, 64, 512, 512, 8, 64
NH, NW = H // K, W // K          # 64, 64
L = NH * NW                      # 4096
NR = 8                           # patch-rows per core
TP = 4                           # row-pairs per core
FD = K * W                       # 4096 free dim per unit
F32 = mybir.dt.float32
BF16 = mybir.dt.bfloat16

_ERF = np.frompyfunc(math.erf, 1, 1)


def _gelu(x):
    x = np.asarray(x, np.float64)
    return 0.5 * x * (1.0 + _ERF(x / math.sqrt(2.0)).astype(np.float64))


def _host_tables(w1k, b1k, w2k, b2k, w1b, b1b, w2b, b2b, conv_w, conv_b):
    """pk/pb via the tiny MLPs plus the folded conv constants."""
    cy = cx = H / 2.0
    max_d = math.sqrt(cy * cy + cx * cx)
    py = np.arange(NH, dtype=np.float64) * K + K / 2.0
    px = np.arange(NW, dtype=np.float64) * K + K / 2.0
    d = np.sqrt((py - cy)[:, None] ** 2 + (px - cx)[None, :] ** 2) / max_d
    d = d.reshape(L, 1)

    pk = _gelu(d @ w1k.astype(np.float64) + b1k) @ w2k.astype(np.float64) + b2k
    pb = (_gelu(d @ w1b.astype(np.float64) + b1b) @ w2b.astype(np.float64) + b2b)[:, 0]

    # g = sum_{dy2,c} bd2[(dy2 c),(dy2' o)] t3 + w2.T @ [pb; 1]
    cw1 = conv_w.astype(np.float64).sum(axis=1)
    w2 = np.stack([np.tile(cw1, 2), np.tile(conv_b.astype(np.float64), 2)]).astype(
        np.float32
    )  # [2, 128]
    bd2 = np.tile(conv_w.T.astype(np.float32), (2, 2))  # [128, 128]
    return pk, pb, w2, bd2


def build_program():
    nc = bacc.Bacc("TRN2", target_bir_lowering=False, debug=False, num_devices=8)
    x_d = nc.dram_tensor("x", [TP, B, 128, FD], BF16, kind="ExternalInput")
    pkr_d = nc.dram_tensor("pkr", [TP, 2, FD], BF16, kind="ExternalInput")
    pkrep0_d = nc.dram_tensor("pkrep0", [128, FD], BF16, kind="ExternalInput")
    pbx_d = nc.dram_tensor("pbx", [2, TP * 128], BF16, kind="ExternalInput")
    w2_d = nc.dram_tensor("w2", [2, 128], BF16, kind="ExternalInput")
    bd2_d = nc.dram_tensor("bd2", [128, 128], BF16, kind="ExternalInput")
    ones2_d = nc.dram_tensor("ones2", [2, 128], BF16, kind="ExternalInput")
    out_d = nc.dram_tensor("out", [TP, B, 128, FD], BF16, kind="ExternalOutput")

    with tile.TileContext(nc) as tc:
        with (
            tc.tile_pool(name="const", bufs=1) as constp,
            tc.tile_pool(name="pkline", bufs=1) as pklinep,
            tc.tile_pool(name="pkrep", bufs=3) as pkrepp,
            tc.tile_pool(name="xbp", bufs=7) as xbp,
            tc.tile_pool(name="prodp", bufs=7) as prodp,
            tc.tile_pool(name="t1p", bufs=2) as t1p,
            tc.tile_pool(name="t3p", bufs=3) as t3p,
            tc.tile_pool(name="gexpp", bufs=3) as gexpp,
            tc.tile_pool(name="gpsum", bufs=3, space="PSUM") as gpsum,
            tc.tile_pool(name="pkpsum", bufs=2, space="PSUM") as pkpsum,
        ):
            # row-pair 0's replicated table ships first on the scalar ring
            pkrep0_t = pkrepp.tile([128, FD], BF16, name="pkrep0")
            nc.scalar.dma_start(pkrep0_t[:], pkrep0_d[:])

            pbx = constp.tile([2, TP * 128], BF16)
            nc.scalar.dma_start(pbx[:], pbx_d[:])
            w2t = constp.tile([2, 128], BF16)
            nc.scalar.dma_start(w2t[:], w2_d[:])
            bd2t = constp.tile([128, 128], BF16)
            nc.scalar.dma_start(bd2t[:], bd2_d[:])
            ones2 = constp.tile([2, 128], BF16)
            nc.scalar.dma_start(ones2[:], ones2_d[:])

            def in_ring(i):
                return nc.sync if i % 2 == 0 else nc.scalar

            def out_ring(i):
                return nc.scalar if i % 2 == 0 else nc.sync

            def build_pkrep(tp):
                """Broadcast pk half-line dy2 across partition half dy2:
                HWDGE line load -> PE ones2-matmul -> PSUM -> ACT copy."""
                pkrep = pkrepp.tile([128, FD], BF16)
                pkl = pklinep.tile([2, FD], BF16)
                (nc.sync if tp % 2 == 0 else nc.scalar).dma_start(
                    pkl[:], pkr_d[tp]
                )
                for ch in range(FD // 512):
                    ps = pkpsum.tile([128, 512], F32)
                    nc.tensor.matmul(
                        ps[:],
                        ones2[:],
                        pkl[:, ch * 512 : (ch + 1) * 512],
                        start=True,
                        stop=True,
                    )
                    nc.scalar.copy(pkrep[:, ch * 512 : (ch + 1) * 512], ps[:])
                return pkrep

            def emit_outmul(st):
                """Deferred modulation+store: the unit's dead prod buffer
                becomes the output buffer.  4-D APs (the 3-D form of this
                broadcast measured slower on HW)."""
                prod, xb, gexp, nh, tp, b, fr, i = st
                o4 = prod.rearrange("p (t2 dyf q) -> p t2 dyf q", t2=nh, dyf=4)
                x4 = xb.rearrange("p (t2 dyf q) -> p t2 dyf q", t2=nh, dyf=4)
                g4 = gexp.rearrange("p (t2 a q) -> p t2 a q", t2=nh, a=1)
                x4b, g4b = bass.broadcast_tensor_aps(x4, g4)
                nc.vector.tensor_tensor(o4, x4b, g4b, op=mybir.AluOpType.mult)
                out_ring(i).dma_start(
                    out_d[tp, b][:, fr].rearrange("p (r w) -> p r w", w=256),
                    prod.rearrange("p (r w) -> p r w", w=256),
                )

            pkreps = {0: pkrep0_t}
            pend = []

            def emit_unit(i, tp, b, h0, nh):
                """One pipeline unit over t2-halves [h0, h0+nh) of image b,
                row-pair tp.  nh=2 is the steady-state full unit; the first
                and last units run as nh=1 halves to shorten ramp/drain."""
                flen = nh * 2048
                fr = slice(h0 * 2048, h0 * 2048 + flen)

                xb = xbp.tile([128, flen], BF16)
                in_ring(i).dma_start(
                    xb.rearrange("p (r w) -> p r w", w=256),
                    x_d[tp, b][:, fr].rearrange("p (r w) -> p r w", w=256),
                )

                # PROD = xb * pkrep[tp]  (2x bf16)
                prod = prodp.tile([128, flen], BF16)
                nc.vector.tensor_tensor(
                    prod[:], xb[:], pkreps[tp][:, fr], op=mybir.AluOpType.mult
                )

                with nc.allow_low_precision("pairwise bf16 tree adds"):
                    # L1: dyf 4 -> 2
                    t1 = t1p.tile([128, flen // 2], BF16)
                    pr4 = prod.rearrange(
                        "p (t2 dyf q) -> p t2 dyf q", t2=nh, dyf=4
                    )
                    t14 = t1.rearrange(
                        "p (t2 dyf q) -> p t2 dyf q", t2=nh, dyf=2
                    )
                    nc.vector.tensor_tensor(
                        t14,
                        pr4[:, :, 0:2, :],
                        pr4[:, :, 2:4, :],
                        op=mybir.AluOpType.add,
                    )
                    # L2: dyf 2 -> 1
                    t3 = t3p.tile([128, flen // 4], BF16)
                    t14b = t1.rearrange(
                        "p (t2 dyf q) -> p t2 dyf q", t2=nh, dyf=2
                    )
                    nc.vector.tensor_tensor(
                        t3.rearrange("p (t2 a q) -> p t2 a q", t2=nh, a=1),
                        t14b[:, :, 0:1, :],
                        t14b[:, :, 1:2, :],
                        op=mybir.AluOpType.add,
                    )

                # g[(dy2' o), (t2 pw)] = sum_dx bd2.T @ t3[:, :, dx]
                #                       + w2.T @ [pb; 1]
                # (dy2-sum, dx-sum and the 1x1 conv share one PSUM group)
                g = gpsum.tile([128, flen // 32], F32)
                t3x = t3.rearrange("p (q dx) -> p q dx", dx=K)
                for j in range(K):
                    nc.tensor.matmul(
                        g[:],
                        bd2t[:],
                        t3x[:, :, j : j + 1],
                        start=(j == 0),
                        stop=False,
                    )
                pb0 = tp * 128 + h0 * 64
                nc.tensor.matmul(
                    g[:],
                    w2t[:],
                    pbx[:, pb0 : pb0 + nh * 64],
                    start=False,
                    stop=True,
                )

                # modulation of the unit THREE back: its gexp chain
                # (DVE L2 -> PE 9 matmuls -> ACT) finished during ~2.5
                # units of DVE mul+L1+L2, so the outmul issues stall-free.
                if len(pend) == 3:
                    emit_outmul(pend.pop(0))

                # cast g to bf16 expanded over dx (dense 8-elem runs)
                gexp = gexpp.tile([128, flen // 4], BF16, tag="gexp")
                ge3 = gexp.rearrange("p (q dx) -> p q dx", dx=K)
                gs3 = g.rearrange("p (q a) -> p q a", a=1)
                ge3b, gs3b = bass.broadcast_tensor_aps(ge3, gs3)
                nc.scalar.copy(ge3b, gs3b)

                pend.append((prod, xb, gexp, nh, tp, b, fr, i))

            units = [(0, 0, 0, 1), (0, 0, 1, 1)]
            for u in range(1, TP * B - 1):
                tp, b = divmod(u, B)
                units.append((tp, b, 0, 2))
            units += [(TP - 1, B - 1, 0, 1), (TP - 1, B - 1, 1, 1)]

            for i, (tp, b, h0, nh) in enumerate(units):
                # build row-pair tp+1's table one pair-slot ahead of use
                if b == 1 and h0 == 0 and tp + 1 < TP:
                    pkreps[tp + 1] = build_pkrep(tp + 1)
                emit_unit(i, tp, b, h0, nh)

            for st in pend:
                emit_outmul(st)

    nc.compile()
    return nc


_PROGRAM = None
LAST_RESULT = None


def make_in_maps(x_bf, pk, pb, w2, bd2):
    """Per-core input dict; x_bf is the full [B,C,H,W] array in bf16."""
    import ml_dtypes

    bf16 = ml_dtypes.bfloat16
    in_maps = []
    ones2 = np.zeros((2, 128), np.float32)
    ones2[0, :64] = 1.0
    ones2[1, 64:] = 1.0
    for i in range(8):
        r0 = i * NR
        # [b, c, tp, t2, dy2, dyf, pw, dx] -> [tp, b, (dy2 c), (t2 dyf pw dx)]
        xs = x_bf[:, :, r0 * K : (r0 + NR) * K, :].reshape(
            B, C, TP, 2, 2, 4, NW, K
        )
        x_core = np.ascontiguousarray(
            xs.transpose(2, 0, 4, 1, 3, 5, 6, 7)
        ).reshape(TP, B, 128, FD)

        # pk rows for this core: [tp, t2, pw, dy2, dyf, dx]
        pkc = (
            pk[r0 * NW : (r0 + NR) * NW]
            .astype(np.float32)
            .reshape(TP, 2, NW, 2, 4, K)
        )
        pkl = (
            np.ascontiguousarray(pkc.transpose(0, 3, 1, 4, 2, 5))
            .reshape(TP, 2, FD)
            .astype(bf16)
        )
        pkrep0 = (
            np.ascontiguousarray(
                np.repeat(pkl[0].astype(np.float32)[:, None, :], 64, axis=1)
            )
            .reshape(128, FD)
            .astype(bf16)
        )

        pbrows = pb[r0 * NW : (r0 + NR) * NW].astype(np.float32).reshape(
            TP, 2, NW
        )
        pbx = np.empty((2, TP, 2, NW), np.float32)
        pbx[0] = pbrows
        pbx[1] = 1.0
        pbx = pbx.reshape(2, TP * 128)

        in_maps.append(
            {
                "x": x_core,
                "pkr": pkl,
                "pkrep0": pkrep0,
                "pbx": pbx.astype(bf16),
                "w2": w2.astype(bf16),
                "bd2": bd2.astype(bf16),
                "ones2": ones2.astype(bf16),
            }
        )
    return in_maps


def kernel(**inputs):
    global _PROGRAM, LAST_RESULT
    import ml_dtypes

    x = np.ascontiguousarray(np.asarray(inputs["x"], dtype=np.float32))
    pk, pb, w2, bd2 = _host_tables(
        *[
            np.asarray(inputs[k], dtype=np.float32)
            for k in (
                "w1k", "b1k", "w2k", "b2k",
                "w1b", "b1b", "w2b", "b2b",
                "conv_w", "conv_b",
            )
        ]
    )
    if _PROGRAM is None:
        _PROGRAM = build_program()
    nc = _PROGRAM

    x_bf = x.astype(ml_dtypes.bfloat16)
    in_maps = make_in_maps(x_bf, pk, pb, w2, bd2)

    conv_w = np.asarray(inputs["conv_w"], np.float64)
    conv_b = np.asarray(inputs["conv_b"], np.float64)

    def _spot_check(out):
        """Verify a sample of patches against the exact host formula;
        catches the rare silent device corruption (bf16 path ~0.5%/elem)."""
        rng = np.random.default_rng(1234)
        worst = 0.0
        for _ in range(32):
            b = int(rng.integers(B))
            ph = int(rng.integers(NH))
            pw = int(rng.integers(NW))
            l = ph * NW + pw
            patch = x[b, :, ph * K : (ph + 1) * K, pw * K : (pw + 1) * K]
            patch = patch.reshape(C, K * K).astype(np.float64)
            feats = patch @ pk[l] + pb[l]
            g = conv_w @ feats + conv_b
            exp = patch * g[:, None]
            got = out[b, :, ph * K : (ph + 1) * K, pw * K : (pw + 1) * K]
            got = got.reshape(C, K * K).astype(np.float64)
            denom = np.linalg.norm(exp) + 1e-30
            worst = max(worst, float(np.linalg.norm(got - exp) / denom))
        return worst

    res = None
    for attempt in range(4):
        try:
            res = run_bass_kernel_spmd(nc, in_maps, list(range(8)))
        except Exception:
            if attempt == 3:
                raise
            continue
        out = np.empty((B, C, H, W), np.float32)
        for i in range(8):
            r0 = i * NR
            dev = res.results[i]["out"].astype(np.float32)
            # [tp, b, (dy2 c), (t2 dyf pw dx)] -> [b, c, rows, w]
            dev = dev.reshape(TP, B, 2, C, 2, 4, NW, K)
            out[:, :, r0 * K : (r0 + NR) * K, :] = dev.transpose(
                1, 3, 0, 4, 2, 5, 6, 7
            ).reshape(B, C, NR * K, W)
        err = _spot_check(out)
        if err < 0.05:
            break
        if attempt == 3:
            raise RuntimeError(f"device output failed spot check ({err:.3f})")
    LAST_RESULT = res
    return out


# revision 21
# speedup vs baseline: 1.0262x; 1.0262x over previous
"""AFPM (adaptive per-patch modulation) kernel for 8 TRN2 NeuronCores.

Reference computation (B=8, C=64, H=W=512, K=8, HID=64):
  - d[l]: normalized distance of each 8x8 patch center from image center
  - pk[l, kk] / pb[l]: tiny MLPs of d (host-precomputable, data-independent)
  - feats[b,c,l] = sum_kk patches[b,c,kk,l] * pk[l,kk] + pb[l]
  - feats2 = conv_w @ feats + conv_b           (1x1 conv over channels)
  - out patches = patches * feats2[:, :, None, :]

Sharding: core i handles patch-rows i*8..i*8+7 for ALL 8 images.

v12 layout: partitions = (dy2, c) -- the TOP/BOTTOM HALF of each patch
rides the partition axis instead of an image pair.  The dy2-sum then
happens inside the PE conv contraction for free (stationary becomes
tile(conv_w.T, (2,2)), summing both partition halves into both output
halves), deleting one DVE tree level.  unit = (row-pair tp, image b),
tp-major; 32 units of [128, free 4096 = (t2, dyf4, pw64, dx8)], 1 MiB
DMA each direction.  HBM floor = 64 MiB / 358 GB/s ~= 187 us.

  DMA  in   : 1 MiB, rings alternate sync/scalar per unit
  DVE  mul  : PROD = xb * PKREP[tp]     flat TT, 2x bf16   (~2.2us)
  DVE  L1   : dyf 4->2 halving add                         (~1.2us)
  DVE  L2   : dyf 2->1, writes into the unit's half of a shared
              pair tile T3 [128, (b2 t2 pw dx)]            (~0.63us)
  PE   g    : per unit-PAIR, 8 accumulating strided matmuls n=256
              over dx slices of T3 with stationary bd2
              (dy2-sum + dx-sum + conv all in one PSUM group)
              (+) w2.T @ [pb; 1]
  ACT  gexp : bf16(g) expanded over dx  [128, 2048] per pair
  DVE  out  : OUT(prod buf) = xb * bcast(gexp), deferred ~4 units
  DMA  out  : 1 MiB from the prod buffer on the opposite ring

NOTE: gpsimd is deliberately UNUSED for tensor work -- Pool TT ops
contend with DVE for SBUF ports and inflate DVE ops 35-90% (measured),
a net loss.  PE/ACT traffic does not measurably slow DVE.

pk tables: only 4 (one per row-pair, shared by all 8 images).  They
ship as [4, 2, 4096] bf16 half-lines; on-device a 2-row ones matmul
broadcasts line dy2 to partition half dy2 (PE -> PSUM -> ACT copy).
Row-pair 0 additionally ships pre-replicated (1 MiB, scalar ring at
t=0) so the first mul isn't gated on the broadcast chain.
"""

import math
import sys

import numpy as np

for _p in ("/opt/trn_rl_repo",):
    if _p not in sys.path:
        sys.path.insert(0, _p)

import concourse.bass as bass
import concourse.tile as tile
from concourse import bacc, mybir
from concourse.bass_utils import run_bass_kernel_spmd

B, C, H, W, K, HID = 8, 64, 512, 512, 8, 64
NH, NW = H // K, W // K          # 64, 64
L = NH * NW                      # 4096
NR = 8                           # patch-rows per core
TP = 4                           # row-pairs per core
FD = K * W                       # 4096 free dim per unit
F32 = mybir.dt.float32
BF16 = mybir.dt.bfloat16

_ERF = np.frompyfunc(math.erf, 1, 1)


def _gelu(x):
    x = np.asarray(x, np.float64)
    return 0.5 * x * (1.0 + _ERF(x / math.sqrt(2.0)).astype(np.float64))


def _host_tables(w1k, b1k, w2k, b2k, w1b, b1b, w2b, b2b, conv_w, conv_b):
    """pk/pb via the tiny MLPs plus the folded conv constants."""
    cy = cx = H / 2.0
    max_d = math.sqrt(cy * cy + cx * cx)
    py = np.arange(NH, dtype=np.float64) * K + K / 2.0
    px = np.arange(NW, dtype=np.float64) * K + K / 2.0
    d = np.sqrt((py - cy)[:, None] ** 2 + (px - cx)[None, :] ** 2) / max_d
    d = d.reshape(L, 1)

    pk = _gelu(d @ w1k.astype(np.float64) + b1k) @ w2k.astype(np.float64) + b2k
    pb = (_gelu(d @ w1b.astype(np.float64) + b1b) @ w2b.astype(np.float64) + b2b)[:, 0]

    # g = sum_{dy2,c} bd2[(dy2 c),(dy2' o)] t3 + w2.T @ [pb; 1]
    cw1 = conv_w.astype(np.float64).sum(axis=1)
    w2 = np.stack([np.tile(cw1, 2), np.tile(conv_b.astype(np.float64), 2)]).astype(
        np.float32
    )  # [2, 128]
    bd2 = np.tile(conv_w.T.astype(np.float32), (2, 2))  # [128, 128]
    return pk, pb, w2, bd2


def build_program():
    nc = bacc.Bacc("TRN2", target_bir_lowering=False, debug=False, num_devices=8)
    x_d = nc.dram_tensor("x", [TP, B, 128, FD], BF16, kind="ExternalInput")
    pkr_d = nc.dram_tensor("pkr", [TP, 2, FD], BF16, kind="ExternalInput")
    pkrep0_d = nc.dram_tensor("pkrep0", [128, FD], BF16, kind="ExternalInput")
    pbx_d = nc.dram_tensor("pbx", [2, TP * 128], BF16, kind="ExternalInput")
    w2_d = nc.dram_tensor("w2", [2, 128], BF16, kind="ExternalInput")
    bd2_d = nc.dram_tensor("bd2", [128, 128], BF16, kind="ExternalInput")
    ones2_d = nc.dram_tensor("ones2", [2, 128], BF16, kind="ExternalInput")
    out_d = nc.dram_tensor("out", [TP, B, 128, FD], BF16, kind="ExternalOutput")

    with tile.TileContext(nc) as tc:
        with (
            tc.tile_pool(name="const", bufs=1) as constp,
            tc.tile_pool(name="pkline", bufs=1) as pklinep,
            tc.tile_pool(name="pkrep", bufs=3) as pkrepp,
            tc.tile_pool(name="xbp", bufs=7) as xbp,
            tc.tile_pool(name="prodp", bufs=7) as prodp,
            tc.tile_pool(name="t1p", bufs=2) as t1p,
            tc.tile_pool(name="t3p", bufs=3) as t3p,
            tc.tile_pool(name="gexpp", bufs=3) as gexpp,
            tc.tile_pool(name="gpsum", bufs=3, space="PSUM") as gpsum,
            tc.tile_pool(name="pkpsum", bufs=2, space="PSUM") as pkpsum,
        ):
            # row-pair 0's replicated table ships first on the scalar ring
            pkrep0_t = pkrepp.tile([128, FD], BF16, name="pkrep0")
            nc.scalar.dma_start(pkrep0_t[:], pkrep0_d[:])

            pbx = constp.tile([2, TP * 128], BF16)
            nc.scalar.dma_start(pbx[:], pbx_d[:])
            w2t = constp.tile([2, 128], BF16)
            nc.scalar.dma_start(w2t[:], w2_d[:])
            bd2t = constp.tile([128, 128], BF16)
            nc.scalar.dma_start(bd2t[:], bd2_d[:])
            ones2 = constp.tile([2, 128], BF16)
            nc.scalar.dma_start(ones2[:], ones2_d[:])

            def in_ring(i):
                return nc.sync if i % 2 == 0 else nc.scalar

            def out_ring(i):
                return nc.scalar if i % 2 == 0 else nc.sync

            def build_pkrep(tp):
                """Broadcast pk half-line dy2 across partition half dy2:
                HWDGE line load -> PE ones2-matmul -> PSUM -> ACT copy."""
                pkrep = pkrepp.tile([128, FD], BF16)
                pkl = pklinep.tile([2, FD], BF16)
                (nc.sync if tp % 2 == 0 else nc.scalar).dma_start(
                    pkl[:], pkr_d[tp]
                )
                for ch in range(FD // 512):
                    ps = pkpsum.tile([128, 512], F32)
                    nc.tensor.matmul(
                        ps[:],
                        ones2[:],
                        pkl[:, ch * 512 : (ch + 1) * 512],
                        start=True,
                        stop=True,
                    )
                    nc.scalar.copy(pkrep[:, ch * 512 : (ch + 1) * 512], ps[:])
                return pkrep

            def emit_outmul(st):
                """Deferred modulation+store: the unit's dead prod buffer
                becomes the output buffer.  4-D APs (the 3-D form of this
                broadcast measured slower on HW)."""
                prod, xb, gexp, nh, tp, b, fr, i = st
                o4 = prod.rearrange("p (t2 dyf q) -> p t2 dyf q", t2=nh, dyf=4)
                x4 = xb.rearrange("p (t2 dyf q) -> p t2 dyf q", t2=nh, dyf=4)
                g4 = gexp.rearrange("p (t2 a q) -> p t2 a q", t2=nh, a=1)
                x4b, g4b = bass.broadcast_tensor_aps(x4, g4)
                nc.vector.tensor_tensor(o4, x4b, g4b, op=mybir.AluOpType.mult)
                out_ring(i).dma_start(
                    out_d[tp, b][:, fr].rearrange("p (r w) -> p r w", w=256),
                    prod.rearrange("p (r w) -> p r w", w=256),
                )

            pkreps = {0: pkrep0_t}
            pend = []

            def emit_unit(i, tp, b, h0, nh):
                """One pipeline unit over t2-halves [h0, h0+nh) of image b,
                row-pair tp.  nh=2 is the steady-state full unit; the first
                and last units run as nh=1 halves to shorten ramp/drain."""
                flen = nh * 2048
                fr = slice(h0 * 2048, h0 * 2048 + flen)

                xb = xbp.tile([128, flen], BF16)
                in_ring(i).dma_start(
                    xb.rearrange("p (r w) -> p r w", w=256),
                    x_d[tp, b][:, fr].rearrange("p (r w) -> p r w", w=256),
                )

                # PROD = xb * pkrep[tp]  (2x bf16)
                prod = prodp.tile([128, flen], BF16)
                nc.vector.tensor_tensor(
                    prod[:], xb[:], pkreps[tp][:, fr], op=mybir.AluOpType.mult
                )

                with nc.allow_low_precision("pairwise bf16 tree adds"):
                    # L1: dyf 4 -> 2
                    t1 = t1p.tile([128, flen // 2], BF16)
                    pr4 = prod.rearrange(
                        "p (t2 dyf q) -> p t2 dyf q", t2=nh, dyf=4
                    )
                    t14 = t1.rearrange(
                        "p (t2 dyf q) -> p t2 dyf q", t2=nh, dyf=2
                    )
                    nc.vector.tensor_tensor(
                        t14,
                        pr4[:, :, 0:2, :],
                        pr4[:, :, 2:4, :],
                        op=mybir.AluOpType.add,
                    )
                    # L2: dyf 2 -> 1
                    t3 = t3p.tile([128, flen // 4], BF16)
                    t14b = t1.rearrange(
                        "p (t2 dyf q) -> p t2 dyf q", t2=nh, dyf=2
                    )
                    nc.vector.tensor_tensor(
                        t3.rearrange("p (t2 a q) -> p t2 a q", t2=nh, a=1),
                        t14b[:, :, 0:1, :],
                        t14b[:, :, 1:2, :],
                        op=mybir.AluOpType.add,
                    )

                # g[(dy2' o), (t2 pw)] = sum_dx bd2.T @ t3[:, :, dx]
                #                       + w2.T @ [pb; 1]
                # (dy2-sum, dx-sum and the 1x1 conv share one PSUM group)
                g = gpsum.tile([128, flen // 32], F32)
                t3x = t3.rearrange("p (q dx) -> p q dx", dx=K)
                for j in range(K):
                    nc.tensor.matmul(
                        g[:],
                        bd2t[:],
                        t3x[:, :, j : j + 1],
                        start=(j == 0),
                        stop=False,
                    )
                pb0 = tp * 128 + h0 * 64
                nc.tensor.matmul(
                    g[:],
                    w2t[:],
                    pbx[:, pb0 : pb0 + nh * 64],
                    start=False,
                    stop=True,
                )

                # modulation of the unit THREE back: its gexp chain
                # (DVE L2 -> PE 9 matmuls -> ACT) finished during ~2.5
                # units of DVE mul+L1+L2, so the outmul issues stall-free.
                if len(pend) == 3:
                    emit_outmul(pend.pop(0))

                # cast g to bf16 expanded over dx (dense 8-elem runs)
                gexp = gexpp.tile([128, flen // 4], BF16, tag="gexp")
                ge3 = gexp.rearrange("p (q dx) -> p q dx", dx=K)
                gs3 = g.rearrange("p (q a) -> p q a", a=1)
                ge3b, gs3b = bass.broadcast_tensor_aps(ge3, gs3)
                nc.scalar.copy(ge3b, gs3b)

                pend.append((prod, xb, gexp, nh, tp, b, fr, i))

            units = []
            for u in range(TP * B):
                tp, b = divmod(u, B)
                units.append((tp, b, 0, 2))

            for i, (tp, b, h0, nh) in enumerate(units):
                # build row-pair tp+1's table one pair-slot ahead of use
                if b == 1 and h0 == 0 and tp + 1 < TP:
                    pkreps[tp + 1] = build_pkrep(tp + 1)
                emit_unit(i, tp, b, h0, nh)

            for st in pend:
                emit_outmul(st)

    nc.compile()
    return nc


_PROGRAM = None
LAST_RESULT = None


def make_in_maps(x_bf, pk, pb, w2, bd2):
    """Per-core input dict; x_bf is the full [B,C,H,W] array in bf16."""
    import ml_dtypes

    bf16 = ml_dtypes.bfloat16
    in_maps = []
    ones2 = np.zeros((2, 128), np.float32)
    ones2[0, :64] = 1.0
    ones2[1, 64:] = 1.0
    for i in range(8):
        r0 = i * NR
        # [b, c, tp, t2, dy2, dyf, pw, dx] -> [tp, b, (dy2 c), (t2 dyf pw dx)]
        xs = x_bf[:, :, r0 * K : (r0 + NR) * K, :].reshape(
            B, C, TP, 2, 2, 4, NW, K
        )
        x_core = np.ascontiguousarray(
            xs.transpose(2, 0, 4, 1, 3, 5, 6, 7)
        ).reshape(TP, B, 128, FD)

        # pk rows for this core: [tp, t2, pw, dy2, dyf, dx]
        pkc = (
            pk[r0 * NW : (r0 + NR) * NW]
            .astype(np.float32)
            .reshape(TP, 2, NW, 2, 4, K)
        )
        pkl = (
            np.ascontiguousarray(pkc.transpose(0, 3, 1, 4, 2, 5))
            .reshape(TP, 2, FD)
            .astype(bf16)
        )
        pkrep0 = (
            np.ascontiguousarray(
                np.repeat(pkl[0].astype(np.float32)[:, None, :], 64, axis=1)
            )
            .reshape(128, FD)
            .astype(bf16)
        )

        pbrows = pb[r0 * NW : (r0 + NR) * NW].astype(np.float32).reshape(
            TP, 2, NW
        )
        pbx = np.empty((2, TP, 2, NW), np.float32)
        pbx[0] = pbrows
        pbx[1] = 1.0
        pbx = pbx.reshape(2, TP * 128)

        in_maps.append(
            {
                "x": x_core,
                "pkr": pkl,
                "pkrep0": pkrep0,
                "pbx": pbx.astype(bf16),
                "w2": w2.astype(bf16),
                "bd2": bd2.astype(bf16),
                "ones2": ones2.astype(bf16),
            }
        )
    return in_maps


def kernel(**inputs):
    global _PROGRAM, LAST_RESULT
    import ml_dtypes

    x = np.ascontiguousarray(np.asarray(inputs["x"], dtype=np.float32))
    pk, pb, w2, bd2 = _host_tables(
        *[
            np.asarray(inputs[k], dtype=np.float32)
            for k in (
                "w1k", "b1k", "w2k", "b2k",
                "w1b", "b1b", "w2b", "b2b",
                "conv_w", "conv_b",
            )
        ]
    )
    if _PROGRAM is None:
        _PROGRAM = build_program()
    nc = _PROGRAM

    x_bf = x.astype(ml_dtypes.bfloat16)
    in_maps = make_in_maps(x_bf, pk, pb, w2, bd2)

    conv_w = np.asarray(inputs["conv_w"], np.float64)
    conv_b = np.asarray(inputs["conv_b"], np.float64)

    def _spot_check(out):
        """Verify a sample of patches against the exact host formula;
        catches the rare silent device corruption (bf16 path ~0.5%/elem)."""
        rng = np.random.default_rng(1234)
        worst = 0.0
        for _ in range(32):
            b = int(rng.integers(B))
            ph = int(rng.integers(NH))
            pw = int(rng.integers(NW))
            l = ph * NW + pw
            patch = x[b, :, ph * K : (ph + 1) * K, pw * K : (pw + 1) * K]
            patch = patch.reshape(C, K * K).astype(np.float64)
            feats = patch @ pk[l] + pb[l]
            g = conv_w @ feats + conv_b
            exp = patch * g[:, None]
            got = out[b, :, ph * K : (ph + 1) * K, pw * K : (pw + 1) * K]
            got = got.reshape(C, K * K).astype(np.float64)
            denom = np.linalg.norm(exp) + 1e-30
            worst = max(worst, float(np.linalg.norm(got - exp) / denom))
        return worst

    res = None
    for attempt in range(4):
        try:
            res = run_bass_kernel_spmd(nc, in_maps, list(range(8)))
        except Exception:
            if attempt == 3:
                raise
            continue
        out = np.empty((B, C, H, W), np.float32)
        for i in range(8):
            r0 = i * NR
            dev = res.results[i]["out"].astype(np.float32)
            # [tp, b, (dy2 c), (t2 dyf pw dx)] -> [b, c, rows, w]
            dev = dev.reshape(TP, B, 2, C, 2, 4, NW, K)
            out[:, :, r0 * K : (r0 + NR) * K, :] = dev.transpose(
                1, 3, 0, 4, 2, 5, 6, 7
            ).reshape(B, C, NR * K, W)
        err = _spot_check(out)
        if err < 0.05:
            break
        if attempt == 3:
            raise RuntimeError(f"device output failed spot check ({err:.3f})")
    LAST_RESULT = res
    return out
